# revision 7
# baseline (speedup 1.0000x reference)
"""MoE feed-forward (8 experts, top-2, D=1024, H=4096) on 8 Trainium2 cores.

Strategy: load-balanced expert-parallel with host-side routing, fp16.
  - Host computes the gating (logits -> top-2 -> softmax) in fp64.
  - The 16384 (token, expert) assignments are packed into 16 fixed-size
    slots — 8 "A" slots of 1152 tokens and 8 "B" slots of 1024 tokens,
    one (A, B) pair per core, each slot holding tokens of a single
    expert — so every core processes exactly 2176 tokens instead of
    padding all cores to the busiest expert (2304+ tokens).
  - Core program: for its A segment then B segment,
        y = gelu(x @ w1[e] + b1[e]) @ w2[e]
    with the second segment's weights streamed into the same SBUF
    buffers while the first segment computes (WAR-tracked by Tile).
  - Matmul scheduling holds one *moving* operand for 8 back-to-back
    matmuls (measured ~35ns penalty per moving-address switch),
    rotating through all 8 PSUM banks:
      m1: moving x[d-block] held while 8 j-stationaries cycle;
      m2: moving h[j-block] held while 8 w2 d-stationaries cycle.
    y comes out d-partitioned ([D, C]); the host transposes.
  - Host combines: out[tok] += p_e * (y + b2[e]).

Self-contained: hardcodes all shapes from the problem spec.
"""

import numpy as np

import concourse.bass as bass
import concourse.mybir as mybir
import concourse.tile as tile
from concourse.bass_utils import run_bass_kernel_spmd

F32 = mybir.dt.float32
F16 = mybir.dt.float16

D_MODEL = 1024
HIDDEN = 4096
N_EXPERTS = 8
TOP_K = 2
DBLK = D_MODEL // 128     # 8
JBLK = HIDDEN // 128      # 32
CT = 384                  # token tile (moving free dim, 3*128)

SLOT_A = 9                # A-slot capacity in 128-token tiles (1152 tokens)
SLOT_B = 8                # B-slot capacity (1024 tokens)


def _seg_widths(tiles128):
    """Split a segment of `tiles128` 128-token tiles into CT-wide (+tail)
    token tiles for the device loop."""
    toks = tiles128 * 128
    widths = [CT] * (toks // CT)
    if toks % CT:
        widths.append(toks % CT)
    return widths


# ---------------------------------------------------------------------------
# Walrus workaround: this container's Tile emits instructions carrying more
# sync waits than the bundled walrus accepts ("Too many sync wait commands").
# Hoist excess waits onto EventSemaphore instructions placed immediately
# before the overloaded instruction (same engine, same block) — semantically
# identical: the engine blocks on each wait in program order.
_CAP_BY_OPCODE = {"EventSemaphore": 2}
_DEFAULT_CAP = 1
_split_counter = [0]


def split_excess_waits(nc):
    for f in nc.m.functions:
        for bb in f.blocks:
            new_insts = []
            changed = False
            for inst in bb.instructions:
                si = inst.sync_info
                waits = list(si.on_wait) if si is not None else []
                cap = _CAP_BY_OPCODE.get(inst.opcode, _DEFAULT_CAP)
                if len(waits) > cap:
                    changed = True
                    excess, keep = waits[:-cap], waits[-cap:]
                    for i in range(0, len(excess), 2):
                        _split_counter[0] += 1
                        new_insts.append(mybir.InstEventSemaphore(
                            name=f"I-waitsplit-{_split_counter[0]}",
                            engine=inst.engine,
                            sync_info=mybir.SyncInfo(
                                on_wait=excess[i:i + 2], on_update=[]),
                        ))
                    inst.sync_info = mybir.SyncInfo(
                        on_wait=keep, on_update=list(si.on_update))
                new_insts.append(inst)
            if changed:
                bb.instructions = new_insts
    return nc


# ---------------------------------------------------------------------------
def build_nc(act=None, reps=1, slot_a=SLOT_A, slot_b=SLOT_B):
    """Per-core 2-segment FFN program: xh [128, 8, C] fp16 -> y [D, C] f32
    (y is d-major; host transposes)."""
    if act is None:
        act = mybir.ActivationFunctionType.Gelu
    SEGS = [_seg_widths(slot_a), _seg_widths(slot_b)]
    C = (slot_a + slot_b) * 128
    nc = bass.Bass()
    xh = nc.dram_tensor("xh", [128, DBLK, C], F16, kind="ExternalInput")
    wsrc = {}
    for s in "AB":
        # w1h[p, c, d, hh] = w1[d*128+p, c*512+hh];  w2h[p, j, dd] = w2[j*128+p, dd]
        wsrc[s] = (
            nc.dram_tensor(f"w1h{s}", [128, 8, DBLK, 512], F16,
                           kind="ExternalInput"),
            nc.dram_tensor(f"w2h{s}", [128, JBLK, D_MODEL], F16,
                           kind="ExternalInput"),
            nc.dram_tensor(f"b1{s}", [HIDDEN], F32, kind="ExternalInput"),
        )
    y = nc.dram_tensor("y", [D_MODEL, C], F32, kind="ExternalOutput")

    # flat tile list: (token offset, width, segment)
    tilespec = []
    off = 0
    for si, widths in enumerate(SEGS):
        for w in widths:
            tilespec.append((off, w, "AB"[si]))
            off += w
    assert off == C

    with tile.TileContext(nc) as tc:
        with (
            tc.tile_pool(name="wpool", bufs=1) as wpool,
            tc.tile_pool(name="xpool", bufs=3) as xpool,
            tc.tile_pool(name="hpool", bufs=1) as hpool,
            tc.tile_pool(name="ypool", bufs=4) as ypool,
            tc.tile_pool(name="psp", bufs=8, space="PSUM") as psp,
        ):
            def whole(_=None):
                b1t = {}
                for s in "AB":
                    b1t[s] = wpool.tile([128, JBLK], F32, tag=f"b1t{s}",
                                        name=f"b1t{s}")
                    nc.sync.dma_start(
                        out=b1t[s][:],
                        in_=wsrc[s][2].ap().rearrange("(b p) -> p b", p=128))
                # prefetch first two token tiles ahead of the weight bulk
                xts = {}
                for t in range(2):
                    off, w, _s = tilespec[t]
                    xt = xpool.tile([128, DBLK, CT], F16, tag="xt",
                                    name="xt")
                    nc.sync.dma_start(
                        out=xt[:, :, 0:w], in_=xh.ap()[:, :, off:off + w])
                    xts[t] = xt
                # segment-A weights; chunked so tile-0 compute overlaps
                w1t = wpool.tile([128, 8, DBLK, 512], F16, tag="w1t")
                for cchunk in range(8):
                    nc.sync.dma_start(
                        out=w1t[:, cchunk], in_=wsrc["A"][0].ap()[:, cchunk])
                w2t = wpool.tile([128, JBLK, D_MODEL], F16, tag="w2t")
                for q in range(4):
                    nc.sync.dma_start(
                        out=w2t[:, 8 * q:8 * q + 8],
                        in_=wsrc["A"][1].ap()[:, 8 * q:8 * q + 8])

                last_a = max(i for i, (_o, _w, s) in enumerate(tilespec)
                             if s == "A")
                for t, (off, w, s) in enumerate(tilespec):
                    if t in xts:
                        xt = xts.pop(t)
                    else:
                        xt = xpool.tile([128, DBLK, CT], F16, tag="xt",
                                        name="xt")
                        nc.sync.dma_start(
                            out=xt[:, :, 0:w],
                            in_=xh.ap()[:, :, off:off + w])

                    # matmul1 + gelu: moving x[d] held for 8 j-stationaries
                    hT = hpool.tile([128, JBLK, CT], F16, tag="hT")
                    for jc in range(JBLK // 8):
                        ps = [psp.tile([128, CT], F32, tag="acc",
                                       name=f"ps{k}") for k in range(8)]
                        for d in range(DBLK):
                            xm = xt[:, d, 0:w]
                            for j8 in range(8):
                                j = jc * 8 + j8
                                cchunk, jj = divmod(j, 4)
                                nc.tensor.matmul(
                                    ps[j8][:, 0:w],
                                    w1t[:, cchunk, d, jj * 128:(jj + 1) * 128],
                                    xm,
                                    start=(d == 0), stop=(d == DBLK - 1))
                        for j8 in range(8):
                            j = jc * 8 + j8
                            nc.scalar.activation(
                                hT[:, j, 0:w], ps[j8][:, 0:w], act,
                                bias=b1t[s][:, j:j + 1])

                    if t == last_a:
                        # stream segment-B w1 while A's last m2 runs (WAR on
                        # m1-A reads is tracked by Tile)
                        for cchunk in range(8):
                            nc.sync.dma_start(
                                out=w1t[:, cchunk],
                                in_=wsrc["B"][0].ap()[:, cchunk])

                    # matmul2: moving h[j] held for 8 w2 d-stationaries
                    qs = [psp.tile([128, CT], F32, tag="acc",
                                   name=f"qs{k}") for k in range(8)]
                    for j in range(JBLK):
                        hm = hT[:, j, 0:w]
                        for db in range(DBLK):
                            nc.tensor.matmul(
                                qs[db][:, 0:w],
                                w2t[:, j, db * 128:(db + 1) * 128],
                                hm,
                                start=(j == 0), stop=(j == JBLK - 1))
                    for db in range(DBLK):
                        yb = ypool.tile([128, CT], F32, tag="yb")
                        nc.vector.tensor_copy(yb[:, 0:w], qs[db][:, 0:w])
                        nc.sync.dma_start(
                            out=y.ap()[db * 128:(db + 1) * 128, off:off + w],
                            in_=yb[:, 0:w])

                    if t == last_a:
                        for q in range(4):
                            nc.sync.dma_start(
                                out=w2t[:, 8 * q:8 * q + 8],
                                in_=wsrc["B"][1].ap()[:, 8 * q:8 * q + 8])

            if reps == 1:
                whole()
            else:
                with tc.For_i(0, reps, 1):
                    whole()
    return nc


# ---------------------------------------------------------------------------
def _gating(x2d, gate_w, gate_b):
    """fp64 host gating; returns per-expert (idx, prob) matching jax top_k
    (ties -> lower index, measure-zero for random inputs)."""
    logits = x2d.astype(np.float64) @ gate_w.astype(np.float64) \
        + gate_b.astype(np.float64)
    i1 = np.argmax(logits, axis=-1)
    n = len(logits)
    ar = np.arange(n)
    v1 = logits[ar, i1]
    l2 = logits.copy()
    l2[ar, i1] = -np.inf
    i2 = np.argmax(l2, axis=-1)
    v2 = l2[ar, i2]
    m = np.maximum(v1, v2)
    e1 = np.exp(v1 - m)
    e2 = np.exp(v2 - m)
    s = e1 + e2
    p1 = (e1 / s)
    p2 = (e2 / s)
    out = []
    for e in range(N_EXPERTS):
        m1 = i1 == e
        m2 = i2 == e
        idx = np.nonzero(m1 | m2)[0]
        prob = np.where(m1, p1, p2)[idx].astype(np.float32)
        out.append((idx, prob))
    return out


def _pack_slots(tiles, slot_a, slot_b):
    """Greedy-pack per-expert 128-token tile counts into 8 A-slots and 8
    B-slots, each slot single-expert. Returns per-expert slot-size lists,
    or None if infeasible."""
    a_left, b_left = 8, 8
    assign = {e: [] for e in range(N_EXPERTS)}
    for e in sorted(range(N_EXPERTS), key=lambda e: -tiles[e]):
        need = tiles[e]
        while need > 0:
            if a_left and (need >= slot_a or not b_left):
                assign[e].append(slot_a)
                a_left -= 1
                need -= slot_a
            elif b_left:
                assign[e].append(slot_b)
                b_left -= 1
                need -= slot_b
            else:
                return None
    return assign


def plan_cores(routes):
    """Map routed tokens to 8 cores x 2 slots. Returns (plan, slot_a,
    slot_b); plan is a per-core dict {'A': (expert, pos_array), 'B': ...}
    where pos indexes into routes[expert], or -1 for padding. Slot sizes
    start at (9, 8) and grow until the packing fits (always terminates)."""
    tiles = [-(-len(routes[e][0]) // 128) for e in range(N_EXPERTS)]
    slot_a, slot_b = SLOT_A, SLOT_B
    for extra in range(256):
        sa = SLOT_A + (extra + 1) // 2
        sb = SLOT_B + extra // 2
        if extra == 0:
            sa, sb = SLOT_A, SLOT_B
        assign = _pack_slots(tiles, sa, sb)
        if assign is not None:
            slot_a, slot_b = sa, sb
            break
    else:
        raise RuntimeError("slot packing failed")
    slot_lists = {"A": [], "B": []}
    for e in range(N_EXPERTS):
        start = 0
        for size in assign[e]:
            s = "A" if size == slot_a else "B"
            n_e = len(routes[e][0])
            pos = np.full(size * 128, -1, dtype=np.int64)
            take = max(0, min(n_e - start, size * 128))
            if take:
                pos[:take] = np.arange(start, start + take)
            slot_lists[s].append((e, pos))
            start += take
    while len(slot_lists["A"]) < 8:
        slot_lists["A"].append((0, np.full(slot_a * 128, -1, np.int64)))
    while len(slot_lists["B"]) < 8:
        slot_lists["B"].append((0, np.full(slot_b * 128, -1, np.int64)))
    plan = [{"A": slot_lists["A"][c], "B": slot_lists["B"][c]}
            for c in range(8)]
    return plan, slot_a, slot_b


def _wlayout(w1e, w2e):
    w1h = np.ascontiguousarray(
        w1e.reshape(DBLK, 128, 8, 512).transpose(1, 2, 0, 3)
    ).astype(np.float16)
    w2h = np.ascontiguousarray(
        w2e.reshape(JBLK, 128, D_MODEL).transpose(1, 0, 2)
    ).astype(np.float16)
    return w1h, w2h


def make_in_maps(x2d, routes, w1, b1, w2, plan, slot_a=SLOT_A,
                 slot_b=SLOT_B):
    """Build the per-core device input dict list."""
    C = (slot_a + slot_b) * 128
    wcache = {}
    in_maps = []
    for core in plan:
        xpad = np.zeros((C, D_MODEL), dtype=np.float32)
        im = {}
        col = 0
        for s, cap in (("A", slot_a * 128), ("B", slot_b * 128)):
            e, pos = core[s]
            valid = pos >= 0
            toks = routes[e][0][pos[valid]]
            xpad[col:col + cap][valid] = x2d[toks]
            if e not in wcache:
                wcache[e] = _wlayout(w1[e], w2[e])
            im[f"w1h{s}"], im[f"w2h{s}"] = wcache[e]
            im[f"b1{s}"] = np.ascontiguousarray(b1[e], dtype=np.float32)
            col += cap
        im["xh"] = np.ascontiguousarray(
            xpad.T.reshape(DBLK, 128, C).transpose(1, 0, 2)
        ).astype(np.float16)
        in_maps.append(im)
    return in_maps


_NC_CACHE = {}


def kernel(x, gate_w, gate_b, w1, b1, w2, b2):
    x = np.asarray(x, dtype=np.float32)
    gate_w = np.asarray(gate_w, dtype=np.float32)
    gate_b = np.asarray(gate_b, dtype=np.float32)
    w1 = np.asarray(w1, dtype=np.float32)
    b1 = np.asarray(b1, dtype=np.float32)
    w2 = np.asarray(w2, dtype=np.float32)
    b2 = np.asarray(b2, dtype=np.float32)

    B, T, D = x.shape
    x2d = x.reshape(-1, D)
    routes = _gating(x2d, gate_w, gate_b)
    plan, slot_a, slot_b = plan_cores(routes)

    key = (slot_a, slot_b)
    if key not in _NC_CACHE:
        nc = build_nc(slot_a=slot_a, slot_b=slot_b)
        split_excess_waits(nc)
        _NC_CACHE[key] = nc
    nc = _NC_CACHE[key]

    in_maps = make_in_maps(x2d, routes, w1, b1, w2, plan, slot_a, slot_b)
    res = run_bass_kernel_spmd(nc, in_maps, core_ids=list(range(N_EXPERTS)))

    out2d = np.zeros((B * T, D_MODEL), dtype=np.float32)
    for c, core in enumerate(plan):
        yT = res.results[c]["y"]        # [D, C]
        col = 0
        for s, cap in (("A", slot_a * 128), ("B", slot_b * 128)):
            e, pos = core[s]
            valid = pos >= 0
            idx_e, prob_e = routes[e]
            toks = idx_e[pos[valid]]
            probs = prob_e[pos[valid]]
            y_e = yT[:, col:col + cap].T[valid] + b2[e]
            out2d[toks] += probs[:, None] * y_e
            col += cap
    return out2d.reshape(B, T, D_MODEL)
